# revision 3
# baseline (speedup 1.0000x reference)
"""Trainium2 Bass kernel v2 for nn_EnhancedRecurrentGCN (K=1 DConv DCRNN stack).

Math (h0 == 0 collapses each DCRNN cell):
    h1 = sig(-a1) * max(tanh(b1), 0)   a1 = x@W1z, b1 = x@W1h      [256->128]
    h2 = sig(-a2) * max(tanh(b2), 0)   a2 = h1@W2z, b2 = h1@W2h    [128->64]
    y  = relu(h2@W3 + b3) @ W4 + b4                                 [64->16->1]

Engine budget per core (12800 nodes): ACT is the wall (3 activation
columns/node = 38400 cols @ 1.2GHz ~= 32us + per-call overhead), so ACT
calls stay at N=1024 and every other engine is kept under it:
 - PE ~35us: L1 8 MM/macro (macro = 2 subtiles = 1024 nodes), L2
   pair-packed 4 MM, L3 pair-packed 1 MM (4 macros accumulate into one
   [128,512] group psum covering 8 subtiles), L4 1 MM/group.
 - DVE ~24us: all intermediates fp16 (2x packed mode), one stt for h1,
   ts+stt for the sigma-form tanh of L2, relu/out drains per group of 8.
 - sigma2(m) is scheduled 2 macros behind its L2 matmuls and FIRST in each
   macro's ACT triple, so the single-buffered zt2 psum drains early and
   refills late -> PSUM fits exactly: zpre 2 + tpre 2 + zt2 2 + ph3 1
   + po 1 = 8 banks.
 - L2 gates: zt2 = [z-block | h-block] with weights [-w2z] and [2*w2h],
   both blocks pair-packed (sub-even feats -> psum parts 0:64, sub-odd ->
   64:128).  One sigmoid call gives [s2 | v2]; tanh(b)+relu folds to
   2*max(v,0.5)-1.
 - x shipped pre-transposed fp16 with contraction halves interleaved per
   chunk so each chunk is ONE dma_start (launch overhead ~0.8us each).
"""

import sys

if "/opt/trn_rl_repo" not in sys.path:
    sys.path.insert(0, "/opt/trn_rl_repo")

from contextlib import ExitStack

import numpy as np

import concourse.mybir as mybir
import concourse.tile as tile
from concourse import bacc
from concourse.bass_utils import run_bass_kernel_spmd

N_CORES = 8
PAD_NODES = 102400
SHARD = PAD_NODES // N_CORES  # 12800
TN = 512
N_SUB = SHARD // TN           # 25
GROUP = 8                     # subtiles per output group (4 macros)
N_GRP = (N_SUB + GROUP - 1) // GROUP  # 4

F32 = mybir.dt.float32
FP16 = mybir.dt.float16
NPDT = np.float16
AF = mybir.ActivationFunctionType
OP = mybir.AluOpType

CHUNKS = [512, 512, 1024, 2048, 2560, 2560, 2560, 1024]
assert sum(CHUNKS) == SHARD

N_SLAB = 12
WCOLS = N_SLAB * 128 + 8  # 12 slabs + w4g[8]


def build_nc():
    macros = [(2 * m, 2) for m in range(N_SUB // 2)]
    if N_SUB % 2:
        macros.append((N_SUB - 1, 1))
    n_mac = len(macros)

    chunks = []
    c = 0
    for w in CHUNKS:
        chunks.append((c, w))
        c += w

    nc = bacc.Bacc(None)

    xpk = nc.declare_dram_parameter("xpk", [128, 2 * SHARD], FP16, isOutput=False)
    wpack = nc.declare_dram_parameter("wpack", [128, WCOLS], FP16, isOutput=False)
    bpack = nc.declare_dram_parameter("bpack", [128, 2], F32, isOutput=False)
    out = nc.declare_dram_parameter("out", [GROUP, TN * N_GRP], F32, isOutput=True)

    with ExitStack() as ctx:
        tc = ctx.enter_context(tile.TileContext(nc, pool_alloc_mode="queue"))
        wp = ctx.enter_context(tc.tile_pool(name="weights", bufs=1))
        xp = ctx.enter_context(tc.tile_pool(name="x", bufs=4))
        ap = ctx.enter_context(tc.tile_pool(name="acts", bufs=4))
        ob = ctx.enter_context(tc.tile_pool(name="outbuf", bufs=1))
        # PSUM: zpre 2 + tpre 2 + zt2 2 + ph3 1 + po 1 = 8 banks
        pz = ctx.enter_context(tc.tile_pool(name="pz", bufs=1, space="PSUM"))
        pt = ctx.enter_context(tc.tile_pool(name="pt", bufs=1, space="PSUM"))
        p2 = ctx.enter_context(tc.tile_pool(name="p2", bufs=1, space="PSUM"))
        p3 = ctx.enter_context(tc.tile_pool(name="p3", bufs=1, space="PSUM"))
        p4 = ctx.enter_context(tc.tile_pool(name="p4", bufs=1, space="PSUM"))

        warm_o = wp.tile([128, 2], FP16, name="warm_o")
        c0 = nc.const_aps.tensor(0.0, (128, 1))
        nc.scalar.activation(warm_o[:, 0:1], c0, AF.Sigmoid)
        nc.scalar.activation(warm_o[:, 1:2], c0, AF.Tanh)

        wpack_sb = wp.tile([128, WCOLS], FP16, name="wpack_sb")

        def wslab(k):
            return wpack_sb[:, 128 * k:128 * (k + 1)]

        w1zn = [wslab(0), wslab(1)]    # -(w1z) contraction halves
        w1h = [wslab(2), wslab(3)]     # w1h contraction halves
        w2ze = [wslab(4), wslab(5)]    # -w2z at cols 64v:64v+64
        w2he = [wslab(6), wslab(7)]    # 2*w2h at cols 64v:64v+64
        w3pg = [wslab(8 + p) for p in range(4)]
        w4g = wpack_sb[:, N_SLAB * 128:N_SLAB * 128 + 8]
        bpack_sb = wp.tile([128, 2], F32, name="bpack_sb")
        b3c = bpack_sb[:, 0:1]
        b4c = bpack_sb[:, 1:2]

        out_sb = ob.tile([GROUP, TN * N_GRP], F32)

        x_tiles = {}
        _boot = []

        def ensure_chunk(ci):
            if ci in x_tiles or ci >= len(chunks):
                return
            c0, cw = chunks[ci]
            xc = xp.tile([128, 2 * cw], FP16, tag="xc", name=f"xc{ci}")
            nc.sync.dma_start(xc[:], xpk[:, 2 * c0:2 * c0 + 2 * cw])
            x_tiles[ci] = (xc, cw)

        def x_slice(s):
            col = s * TN
            ci = next(k for k, (c0, cw) in enumerate(chunks)
                      if c0 <= col < c0 + cw)
            ensure_chunk(ci)
            ensure_chunk(ci + 1)
            off = col - chunks[ci][0]
            xc, cw = x_tiles[ci]
            return (xc[:, off:off + TN], xc[:, cw + off:cw + off + TN])

        st = {}

        def stage_a(mi):
            """L1 matmuls for macro mi -> zpre, tpre psum tiles."""
            s0, nsub = macros[mi]
            mw = nsub * TN
            zpre = pz.tile([128, mw], F32, tag="zpre", name=f"zp{mi}")
            tpre = pt.tile([128, mw], F32, tag="tpre", name=f"tp{mi}")
            for i in range(nsub):
                xa, xb = x_slice(s0 + i)
                d = slice(i * TN, (i + 1) * TN)
                nc.tensor.matmul(zpre[:, d], w1zn[0], xa, start=True,
                                 stop=False, skip_group_check=True)
                nc.tensor.matmul(zpre[:, d], w1zn[1], xb, start=False,
                                 stop=True, skip_group_check=True)
            for i in range(nsub):
                xa, xb = x_slice(s0 + i)
                d = slice(i * TN, (i + 1) * TN)
                nc.tensor.matmul(tpre[:, d], w1h[0], xa, start=True,
                                 stop=False, skip_group_check=True)
                nc.tensor.matmul(tpre[:, d], w1h[1], xb, start=False,
                                 stop=True, skip_group_check=True)
            st[mi] = {}
            st[mi]["zpre"], st[mi]["tpre"] = zpre, tpre

        def stage_b1(mi):
            """L1 activations, h1, L2 matmuls -> zt2."""
            s0, nsub = macros[mi]
            mw = nsub * TN
            d = st[mi]
            s1 = ap.tile([128, mw], FP16, tag="s1", name=f"s1_{mi}")
            u1 = ap.tile([128, mw], FP16, tag="u1", name=f"u1_{mi}")
            h1 = ap.tile([128, mw], FP16, tag="h1", name=f"h1_{mi}")
            for lo, hi in ([(i * TN, (i + 1) * TN) for i in range(nsub)]
                           if mi == 0 else [(0, mw)]):
                sl = slice(lo, hi)
                nc.scalar.activation(s1[:, sl], d["zpre"][:, sl], AF.Sigmoid)
                nc.scalar.activation(u1[:, sl], d["tpre"][:, sl], AF.Tanh)
                nc.vector.scalar_tensor_tensor(h1[:, sl], u1[:, sl], 0.0,
                                               s1[:, sl],
                                               op0=OP.max, op1=OP.mult)
            if nsub == 2:
                zt2 = p2.tile([128, 2 * TN], F32, tag="zt2", name=f"zt2_{mi}")
                for blk, wemb in ((0, w2ze), (1, w2he)):
                    dst = zt2[:, blk * TN:(blk + 1) * TN]
                    for i in range(nsub):
                        nc.tensor.matmul(dst, wemb[i],
                                         h1[:, i * TN:(i + 1) * TN],
                                         start=(i == 0), stop=(i == nsub - 1),
                                         skip_group_check=True)
            else:
                # tail: z -> parts 0:64 (w2ze[0]), h -> parts 64:128
                # (w2he[1]) in one 512-col block, halving the sigma2 call
                zt2 = p2.tile([128, TN], F32, tag="zt2", name=f"zt2_{mi}")
                nc.tensor.matmul(zt2[:], w2ze[0], h1[:], start=True,
                                 stop=False, skip_group_check=True)
                nc.tensor.matmul(zt2[:], w2he[1], h1[:], start=False,
                                 stop=True, skip_group_check=True)
            d["zt2"] = zt2

        def stage_b2(mi):
            """sigma2, combine, L3, and group tail (relu, L4, out)."""
            d = st[mi]
            p = mi % 4
            g = mi // 4
            g_last = min(4 * g + 4, n_mac) - 1
            nsub2 = macros[mi][1]
            h2 = ap.tile([128, TN], FP16, tag="h2", name=f"h2_{mi}")
            if nsub2 == 2:
                sv2 = ap.tile([128, 2 * TN], FP16, tag="sv2", name=f"sv2_{mi}")
                nc.scalar.activation(sv2[:], d["zt2"][:], AF.Sigmoid)
                # t+ = 2*max(v, 0.5) - 1 ;  h2 = s2 * t+
                pp = ap.tile([128, TN], FP16, tag="pp", name=f"pp_{mi}")
                nc.vector.tensor_scalar(pp[:], sv2[:, TN:2 * TN], 0.5, 2.0,
                                        op0=OP.max, op1=OP.mult)
                nc.vector.scalar_tensor_tensor(h2[:], pp[:], 1.0,
                                               sv2[:, 0:TN],
                                               op0=OP.subtract, op1=OP.mult)
            else:
                sv2 = ap.tile([128, TN], FP16, tag="sv2", name=f"sv2_{mi}")
                nc.scalar.activation(sv2[:], d["zt2"][:], AF.Sigmoid)
                nc.vector.memset(h2[64:128, :], 0)
                pp = ap.tile([64, TN], FP16, tag="pp", name=f"pp_{mi}")
                nc.vector.tensor_scalar(pp[:], sv2[64:128, :], 0.5, 2.0,
                                        op0=OP.max, op1=OP.mult)
                nc.vector.scalar_tensor_tensor(h2[0:64, :], pp[:], 1.0,
                                               sv2[0:64, :],
                                               op0=OP.subtract, op1=OP.mult)
            if p == 0:
                d["h3g"] = p3.tile([128, TN], F32, tag="h3g", name=f"h3g{g}")
            else:
                d["h3g"] = st[mi - 1]["h3g"]
            nc.tensor.matmul(d["h3g"][:], w3pg[p], h2[:],
                             start=(p == 0), stop=(mi == g_last),
                             skip_group_check=True)
            if mi == g_last:
                h3s = ap.tile([128, TN], FP16, tag="h3s", name=f"h3s{g}")
                nc.vector.tensor_scalar(h3s[:], d["h3g"][:], b3c, 0.0,
                                        op0=OP.add, op1=OP.max)
                opre = p4.tile([GROUP, TN], F32, tag="opre", name=f"op{g}")
                nc.tensor.matmul(opre[:], w4g, h3s[:], start=True, stop=True,
                                 skip_group_check=True)
                gs = sum(macros[k][1] for k in range(4 * g, g_last + 1))
                nc.vector.tensor_scalar(
                    out_sb[0:gs, g * TN:(g + 1) * TN], opre[0:gs, :],
                    b4c[0:gs], None, op0=OP.add)
                nc.sync.dma_start(out[0:gs, g * TN:(g + 1) * TN],
                                  out_sb[0:gs, g * TN:(g + 1) * TN])
            for k in ("zpre", "tpre", "zt2"):
                d.pop(k, None)

        # critical launch order: ch0, wpack, ch1 (stage_a(0) triggers ch0/ch1
        # lazily, but we pre-issue them around the weight DMA here)
        ensure_chunk(0)
        nc.sync.dma_start(wpack_sb[:], wpack[:])
        ensure_chunk(1)
        nc.sync.dma_start(bpack_sb[:], bpack[:])

        # software pipeline: sigma2 lags its L2 matmuls by 2 macros
        for mi in range(n_mac):
            stage_a(mi)
            if mi - 3 >= 0:
                stage_b2(mi - 3)
            if mi - 1 >= 0:
                stage_b1(mi - 1)
        stage_b2(n_mac - 3)
        stage_b1(n_mac - 1)
        stage_b2(n_mac - 2)
        stage_b2(n_mac - 1)

    nc.compile()
    return nc


_NC_CACHE = {}


def _get_nc():
    if "nc" not in _NC_CACHE:
        _NC_CACHE["nc"] = build_nc()
    return _NC_CACHE["nc"]


def make_in_maps(x, w_z1, b_z1, w_r1, b_r1, w_h1, b_h1,
                 w_z2, b_z2, w_r2, b_r2, w_h2, b_h2,
                 w_lin1, b_lin1, w_lin2, b_lin2):
    f = np.float32
    for b in (b_z1, b_h1, b_z2, b_h2):
        assert not np.any(np.asarray(b)), "kernel assumes zero gate biases"
    w1z = np.asarray((np.asarray(w_z1)[0, 0] + np.asarray(w_z1)[1, 0])[:256], f)
    w1h = np.asarray((np.asarray(w_h1)[0, 0] + np.asarray(w_h1)[1, 0])[:256], f)
    w2z = np.asarray((np.asarray(w_z2)[0, 0] + np.asarray(w_z2)[1, 0])[:128], f)
    w2h = np.asarray((np.asarray(w_h2)[0, 0] + np.asarray(w_h2)[1, 0])[:128], f)
    w3 = np.asarray(w_lin1, f)
    w4 = np.asarray(w_lin2, f)

    wpk = np.zeros((128, WCOLS), f)
    wpk[:, 0:128] = -w1z[0:128]
    wpk[:, 128:256] = -w1z[128:256]
    wpk[:, 256:384] = w1h[0:128]
    wpk[:, 384:512] = w1h[128:256]
    for v in range(2):
        wpk[:, 128 * (4 + v) + 64 * v:128 * (4 + v) + 64 * v + 64] = -w2z
        wpk[:, 128 * (6 + v) + 64 * v:128 * (6 + v) + 64 * v + 64] = 2.0 * w2h
    for p in range(4):
        base = 128 * (8 + p)
        wpk[0:64, base + 32 * p:base + 32 * p + 16] = w3
        wpk[64:128, base + 32 * p + 16:base + 32 * p + 32] = w3
    for p in range(4):
        for j in range(2):
            r0 = 32 * p + 16 * j
            wpk[r0:r0 + 16, N_SLAB * 128 + 2 * p + j] = w4[:, 0]
    bpk = np.zeros((128, 2), f)
    bpk[:, 0] = np.tile(np.asarray(b_lin1, f), 8)
    bpk[:, 1] = np.asarray(b_lin2, f).reshape(-1)[0]

    common = {"wpack": wpk.astype(NPDT), "bpack": bpk}

    x = np.asarray(x, f)
    n = x.shape[0]
    xpad = np.zeros((N_CORES * SHARD, 256), f)
    xpad[:n] = x
    shards = xpad.reshape(N_CORES, SHARD, 256)
    in_maps = []
    for i in range(N_CORES):
        xt = np.ascontiguousarray(shards[i].T).astype(NPDT)  # [256, SHARD]
        xpk_i = np.empty((128, 2 * SHARD), NPDT)
        c = 0
        for cw in CHUNKS:
            xpk_i[:, 2 * c:2 * c + cw] = xt[0:128, c:c + cw]
            xpk_i[:, 2 * c + cw:2 * c + 2 * cw] = xt[128:256, c:c + cw]
            c += cw
        in_maps.append(dict(common, xpk=np.ascontiguousarray(xpk_i)))
    return in_maps


def unscramble(res):
    full = np.empty(N_CORES * SHARD, np.float32)
    for i in range(N_CORES):
        o = res[i]
        for g in range(N_GRP):
            gs = min(GROUP, N_SUB - g * GROUP)
            for j in range(gs):
                s = g * GROUP + j
                full[i * SHARD + s * TN:i * SHARD + (s + 1) * TN] = \
                    o[j, g * TN:(g + 1) * TN]
    return full


def kernel(x, edge_index=None, edge_weight=None,
           w_z1=None, b_z1=None, w_r1=None, b_r1=None, w_h1=None, b_h1=None,
           w_z2=None, b_z2=None, w_r2=None, b_r2=None, w_h2=None, b_h2=None,
           w_lin1=None, b_lin1=None, w_lin2=None, b_lin2=None):
    in_maps = make_in_maps(x, w_z1, b_z1, w_r1, b_r1, w_h1, b_h1,
                           w_z2, b_z2, w_r2, b_r2, w_h2, b_h2,
                           w_lin1, b_lin1, w_lin2, b_lin2)
    nc = _get_nc()
    res = run_bass_kernel_spmd(nc, in_maps, list(range(N_CORES))).results
    n = np.asarray(x).shape[0]
    full = unscramble([res[i]["out"] for i in range(N_CORES)])
    return np.ascontiguousarray(full[:n].reshape(n, 1).astype(np.float32))
